# revision 24
# baseline (speedup 1.0000x reference)
"""Trainium2 Bass kernel: batched soft 3-SAT circuit evaluation.

out[b, c] = 1 - prod_k z[c,k],  z = (sign>0 ? 1-x : x)[idx],
x = sigmoid(emb[0]).  Every batch row is identical (input_idx is all
zeros, the embedding has a single row, and jnp.take clamps OOB), so the
device computes each clause result once and broadcast-writes the rows.

Sharding: clauses split across 8 NeuronCores (5250 each, padded 5376).
Host work is index-layout prep only (fold sign into a combined table
index, pad, order literals chunk-major, wrap into the 16-partition
GPSIMD gather layout) plus concatenation of per-core outputs.

Per-core device pipeline (H = 4 column chunks of 1344 cols):
  prologue (4 col-quarters, two HWDGE rings): broadcast-load emb row
    into raw[128, NV]; ACT sigmoid -> x table half; DVE (x*-1)+1 ->
    1-x table half.  Combined table tab[128, 2*NV].
  per chunk h:
    - GPSIMD ap_gather: z[128, 512] literals (8 Q7 groups x 168 clauses)
    - DVE: r = 1 - z0*z1*z2  [128, 168] (replicated within each
      16-partition group)
    - PE: per group g a [K=16]x[M=128]x[N=168] matmul with lhsT=1/16
      broadcasts group g's row into all 128 partitions of PSUM (bitwise
      exact: sum of 16 identical values * 1/16)
    - ACT: copy PSUM -> SBUF bcast tile [128, 8*168]
    - 8 row-block DMAs bcast -> out[128b:128b+128, 1344h:1344h+1344]
      (5.4KB descriptors), alternating the sync/scalar HWDGE rings.
"""

import numpy as np

NV = 10000
C_TOTAL = 42000
KLIT = 3
B = 1024
NCORES = 8
C_CORE = C_TOTAL // NCORES     # 5250
GROUPS = 8                     # Q7 cores / 16-partition groups
C_PAD = 5376                   # padded clauses per core
# uneven column chunks: small first chunk fills the pipeline early
CPGS = [84, 196, 196, 196]     # clauses per (group, chunk)
H = len(CPGS)
C_CHUNKS = [8 * c for c in CPGS]          # output cols per chunk
C_OFFS = [sum(C_CHUNKS[:h]) for h in range(H)]
LPCS = [c * KLIT for c in CPGS]           # real literals per (g, chunk)
LPC_PADS = [-(-l // 32) * 32 for l in LPCS]   # pad to 32 (2-col align)
COLS_HS = [l // 16 for l in LPC_PADS]     # idx cols per chunk
COL_OFFS = [sum(COLS_HS[:h]) for h in range(H)]
IDX_COLS = sum(COLS_HS)
PBLK = 256                     # PSUM cols reserved per group block

_CACHE = {}


def _build():
    import concourse.bass as bass
    import concourse.tile as tile
    from concourse import bacc, mybir
    from contextlib import ExitStack

    f32 = mybir.dt.float32
    AF = mybir.ActivationFunctionType
    OP = mybir.AluOpType

    nc = bacc.Bacc("TRN2", target_bir_lowering=False, debug=False,
                   num_devices=NCORES)
    emb_d = nc.dram_tensor("emb", [1, NV], f32, kind="ExternalInput")
    idx_d = nc.dram_tensor("idxw", [128, IDX_COLS], mybir.dt.int16,
                           kind="ExternalInput")
    out_d = nc.dram_tensor("out", [B, C_PAD], f32, kind="ExternalOutput")

    with tile.TileContext(nc) as tc, ExitStack() as ctx:
        const = ctx.enter_context(tc.tile_pool(name="const", bufs=1))
        work = ctx.enter_context(tc.tile_pool(name="work", bufs=2))
        psum = ctx.enter_context(
            tc.tile_pool(name="psum", bufs=2, space="PSUM"))

        idx_sb = const.tile([128, IDX_COLS], mybir.dt.int16)

        # selector E[:, g, :]: E[k, g, m] = 1/16 iff k//16 == g; matmul
        # with it averages each group's 16 identical partition rows into
        # all 128 output partitions (bitwise exact).
        sel = const.tile([128, GROUPS, 128], f32)
        nc.vector.memset(sel[:], 1.0 / 16.0)
        # keep 1/16 only where 0 <= p - 16g <= 15, i.e. g == p//16
        nc.gpsimd.affine_select(sel[:, :, :], sel[:, :, :],
                                pattern=[[-16, GROUPS], [0, 128]],
                                compare_op=OP.is_ge, fill=0.0,
                                base=0, channel_multiplier=1)
        nc.gpsimd.affine_select(sel[:, :, :], sel[:, :, :],
                                pattern=[[16, GROUPS], [0, 128]],
                                compare_op=OP.is_ge, fill=0.0,
                                base=15, channel_multiplier=-1)

        raw = const.tile([128, NV], f32)
        tab = const.tile([128, 2 * NV], f32)
        rings = [nc.sync, nc.scalar]
        NQ = 8
        q = NV // NQ
        # interleave the broadcast-load eighths across both rings so the
        # first one lands early and ACT streams behind the DMAs
        for c in range(NQ):
            rings[c % 2].dma_start(
                out=raw[:, c * q:(c + 1) * q],
                in_=bass.AP(tensor=emb_d, offset=c * q,
                            ap=[[0, 128], [1, q]]))
        nc.scalar.dma_start(out=idx_sb[:], in_=idx_d[:, :])
        for c in range(NQ):
            sl = slice(c * q, (c + 1) * q)
            xs = slice(NV + c * q, NV + (c + 1) * q)
            nc.scalar.activation(tab[:, xs], raw[:, sl], AF.Sigmoid)
            # 1 - x on DVE, overlaps ACT of the next eighth
            nc.vector.tensor_scalar(tab[:, sl], tab[:, xs], -1.0, 1.0,
                                    OP.mult, OP.add)

        for h in range(H):
            CPG, LPC, LPC_PAD = CPGS[h], LPCS[h], LPC_PADS[h]
            C_CHUNK, C_OFF = C_CHUNKS[h], C_OFFS[h]
            z = work.tile([128, max(LPC_PADS)], f32, tag="z")
            nc.gpsimd.ap_gather(
                z[:, 0:LPC_PAD], tab[:],
                idx_sb[:, COL_OFFS[h]:COL_OFFS[h] + COLS_HS[h]],
                channels=128, num_elems=2 * NV, d=1, num_idxs=LPC_PAD)

            p01 = work.tile([128, max(CPGS)], f32, tag="p01")
            nc.vector.tensor_tensor(p01[:, 0:CPG], z[:, 0:LPC:3],
                                    z[:, 1:LPC:3], OP.mult)
            r = work.tile([128, max(CPGS)], f32, tag="r")
            # r = ((p01 * -1) * z2) + 1 = 1 - z0 z1 z2
            nc.vector.scalar_tensor_tensor(r[:, 0:CPG], p01[:, 0:CPG],
                                           -1.0, z[:, 2:LPC:3],
                                           OP.mult, OP.mult)
            nc.vector.tensor_scalar_add(r[:, 0:CPG], r[:, 0:CPG], 1.0)

            # PE broadcast: group g's (16-replicated) row -> all 128
            # partitions.  sum over the 16 identical values * 1/16 is
            # bitwise exact.
            P = psum.tile([128, GROUPS, PBLK], f32, tag="P")
            for g in range(GROUPS):
                nc.tensor.matmul(P[:, g, 0:CPG], sel[:, g, :],
                                 r[:, 0:CPG], start=True, stop=True)
            # pack the 8 group blocks contiguously so output descriptors
            # are C_CHUNK*4 bytes
            bcast = work.tile([128, GROUPS * max(CPGS)], f32, tag="bcast")
            bt = bcast[:]
            prow = bt.ap[0][0]
            bview = bass.AP(tensor=bt.tensor, offset=bt.offset,
                            ap=[[prow, 128], [CPG, GROUPS], [1, CPG]])
            nc.scalar.activation(bview, P[:, :, 0:CPG], AF.Copy)

            # 8 row-block output DMAs, 128 rows each, spread across both
            # HWDGE rings
            bap = bass.AP(tensor=bt.tensor, offset=bt.offset,
                          ap=[[prow, 128], [1, C_CHUNK]])
            for blk in range(8):
                dst = bass.AP(tensor=out_d,
                              offset=blk * 128 * C_PAD + C_OFF,
                              ap=[[C_PAD, 128], [1, C_CHUNK]])
                rings[blk % 2].dma_start(out=dst, in_=bap)
    nc.compile()
    return nc


def _prep_indices(clause_idx, clause_sign):
    """Per-core wrapped int16 combined-index arrays [128, IDX_COLS].

    Literal order per group g: chunk-major — for chunk h, group g owns
    core clauses [C_CHUNK*h + CPG*g, C_CHUNK*h + CPG*(g+1)), padded to
    LPC_PAD literals per (group, chunk) block.
    """
    idx2 = clause_idx.astype(np.int32) + NV * (clause_sign <= 0.0)
    idx2 = idx2.astype(np.int16)
    per_core = []
    for c in range(NCORES):
        cl = idx2[c * C_CORE:(c + 1) * C_CORE]            # [5250, 3]
        buf = np.zeros((C_PAD, KLIT), dtype=np.int16)
        buf[:cl.shape[0]] = cl
        # group g's stream = concat over chunks of its padded block
        gs = np.zeros((GROUPS, IDX_COLS * 16), dtype=np.int16)
        for h in range(H):
            blk = buf[C_OFFS[h]:C_OFFS[h] + C_CHUNKS[h]]  # [8*CPG, 3]
            blk = blk.reshape(GROUPS, LPCS[h])
            o = COL_OFFS[h] * 16
            gs[:, o:o + LPCS[h]] = blk
        # wrap: literal j at partition 16g + j%16, col j//16
        w = (gs.reshape(GROUPS, IDX_COLS, 16)
               .transpose(0, 2, 1)
               .reshape(128, IDX_COLS))
        per_core.append(np.ascontiguousarray(w))
    return per_core


def _ensure_ntff_hook():
    """The agent image lacks antenv.axon_hooks; synthesize it so
    run_bass_kernel_spmd(trace=True) can capture NTFF profiles."""
    import sys, types
    try:
        from antenv import axon_hooks  # noqa: F401
        return
    except ImportError:
        pass
    m = types.ModuleType("antenv.axon_hooks")
    _hook = [None]
    m.set_axon_ntff_profile_hook = lambda h: _hook.__setitem__(0, h)
    m.get_axon_ntff_profile_hook = lambda: _hook[0]
    sys.modules["antenv.axon_hooks"] = m
    import antenv
    antenv.axon_hooks = m
    from trn_agent_boot.trn_boot import _ntff_profile_via_ctypes
    m.set_axon_ntff_profile_hook(
        _ntff_profile_via_ctypes("/opt/axon/libaxon_pjrt.so"))


def _run(emb, idx_cores, trace=False):
    from concourse.bass_utils import run_bass_kernel_spmd
    if trace:
        _ensure_ntff_hook()
    if "prog" not in _CACHE:
        _CACHE["prog"] = _build()
    nc = _CACHE["prog"]
    in_maps = [{"emb": emb, "idxw": idx_cores[c]} for c in range(NCORES)]
    return run_bass_kernel_spmd(nc, in_maps, list(range(NCORES)),
                                trace=trace)


def kernel(input_idx=None, emb_weight=None, clause_idx=None,
           clause_sign=None, _trace=False, _want_results=False):
    emb = np.ascontiguousarray(np.asarray(emb_weight, dtype=np.float32))
    cidx = np.asarray(clause_idx, dtype=np.int32)
    csgn = np.asarray(clause_sign, dtype=np.float32)
    idx_cores = _prep_indices(cidx, csgn)
    res = _run(emb, idx_cores, trace=_trace)
    full = np.empty((B, C_TOTAL), dtype=np.float32)
    for c in range(NCORES):
        full[:, c * C_CORE:(c + 1) * C_CORE] = \
            res.results[c]["out"][:, :C_CORE]
    if _want_results:
        return full, res
    return full


# revision 26
# speedup vs baseline: 1.0456x; 1.0456x over previous
"""Trainium2 Bass kernel: batched soft 3-SAT circuit evaluation.

out[b, c] = 1 - prod_k z[c,k],  z = (sign>0 ? 1-x : x)[idx],
x = sigmoid(emb[0]).  Every batch row is identical (input_idx is all
zeros, the embedding has a single row, and jnp.take clamps OOB), so the
device computes each clause result once and broadcast-writes the rows.

Sharding: clauses split across 8 NeuronCores (5250 each, padded 5376).
Host work is index-layout prep only (fold sign into a combined table
index, pad, order literals chunk-major, wrap into the 16-partition
GPSIMD gather layout) plus concatenation of per-core outputs.

Per-core device pipeline (H = 4 column chunks of 1344 cols):
  prologue (4 col-quarters, two HWDGE rings): broadcast-load emb row
    into raw[128, NV]; ACT sigmoid -> x table half; DVE (x*-1)+1 ->
    1-x table half.  Combined table tab[128, 2*NV].
  per chunk h:
    - GPSIMD ap_gather: z[128, 512] literals (8 Q7 groups x 168 clauses)
    - DVE: r = 1 - z0*z1*z2  [128, 168] (replicated within each
      16-partition group)
    - PE: per group g a [K=16]x[M=128]x[N=168] matmul with lhsT=1/16
      broadcasts group g's row into all 128 partitions of PSUM (bitwise
      exact: sum of 16 identical values * 1/16)
    - ACT: copy PSUM -> SBUF bcast tile [128, 8*168]
    - 8 row-block DMAs bcast -> out[128b:128b+128, 1344h:1344h+1344]
      (5.4KB descriptors), alternating the sync/scalar HWDGE rings.
"""

import numpy as np

NV = 10000
C_TOTAL = 42000
KLIT = 3
B = 1024
NCORES = 8
C_CORE = C_TOTAL // NCORES     # 5250
GROUPS = 8                     # Q7 cores / 16-partition groups
C_PAD = 5376                   # padded clauses per core
CPGS = [168, 168, 168, 168]    # clauses per (group, chunk)
H = len(CPGS)
C_CHUNKS = [8 * c for c in CPGS]          # output cols per chunk
C_OFFS = [sum(C_CHUNKS[:h]) for h in range(H)]
LPCS = [c * KLIT for c in CPGS]           # real literals per (g, chunk)
LPC_PADS = [-(-l // 32) * 32 for l in LPCS]   # pad to 32 (2-col align)
COLS_HS = [l // 16 for l in LPC_PADS]     # idx cols per chunk
COL_OFFS = [sum(COLS_HS[:h]) for h in range(H)]
IDX_COLS = sum(COLS_HS)
PBLK = 256                     # PSUM cols reserved per group block

_CACHE = {}


def _build():
    import concourse.bass as bass
    import concourse.tile as tile
    from concourse import bacc, mybir
    from contextlib import ExitStack

    f32 = mybir.dt.float32
    AF = mybir.ActivationFunctionType
    OP = mybir.AluOpType

    nc = bacc.Bacc("TRN2", target_bir_lowering=False, debug=False,
                   num_devices=NCORES)
    emb_d = nc.dram_tensor("emb", [1, NV], f32, kind="ExternalInput")
    idx_d = nc.dram_tensor("idxw", [128, IDX_COLS], mybir.dt.int16,
                           kind="ExternalInput")
    out_d = nc.dram_tensor("out", [B, C_PAD], f32, kind="ExternalOutput")

    with tile.TileContext(nc) as tc, ExitStack() as ctx:
        const = ctx.enter_context(tc.tile_pool(name="const", bufs=1))
        work = ctx.enter_context(tc.tile_pool(name="work", bufs=2))
        psum = ctx.enter_context(
            tc.tile_pool(name="psum", bufs=2, space="PSUM"))

        idx_sb = const.tile([128, IDX_COLS], mybir.dt.int16)

        # selector E[:, g, :]: E[k, g, m] = 1/16 iff k//16 == g; matmul
        # with it averages each group's 16 identical partition rows into
        # all 128 output partitions (bitwise exact).
        sel = const.tile([128, GROUPS, 128], f32)
        nc.vector.memset(sel[:], 1.0 / 16.0)
        # keep 1/16 only where 0 <= p - 16g <= 15, i.e. g == p//16
        nc.gpsimd.affine_select(sel[:, :, :], sel[:, :, :],
                                pattern=[[-16, GROUPS], [0, 128]],
                                compare_op=OP.is_ge, fill=0.0,
                                base=0, channel_multiplier=1)
        nc.gpsimd.affine_select(sel[:, :, :], sel[:, :, :],
                                pattern=[[16, GROUPS], [0, 128]],
                                compare_op=OP.is_ge, fill=0.0,
                                base=15, channel_multiplier=-1)

        raw = const.tile([128, NV], f32)
        tab = const.tile([128, 2 * NV], f32)
        rings = [nc.sync, nc.scalar]
        NQ = 8
        q = NV // NQ
        # all broadcast-load eighths on the sync ring: the scalar ring
        # shares a sequencer with ACT, and DMA dispatches there would
        # delay the first sigmoid
        for c in range(NQ):
            nc.sync.dma_start(
                out=raw[:, c * q:(c + 1) * q],
                in_=bass.AP(tensor=emb_d, offset=c * q,
                            ap=[[0, 128], [1, q]]))
        nc.sync.dma_start(out=idx_sb[:], in_=idx_d[:, :])
        for c in range(NQ):
            sl = slice(c * q, (c + 1) * q)
            xs = slice(NV + c * q, NV + (c + 1) * q)
            nc.scalar.activation(tab[:, xs], raw[:, sl], AF.Sigmoid)
            # 1 - x on DVE, overlaps ACT of the next eighth
            nc.vector.tensor_scalar(tab[:, sl], tab[:, xs], -1.0, 1.0,
                                    OP.mult, OP.add)

        for h in range(H):
            CPG, LPC, LPC_PAD = CPGS[h], LPCS[h], LPC_PADS[h]
            C_CHUNK, C_OFF = C_CHUNKS[h], C_OFFS[h]
            z = work.tile([128, max(LPC_PADS)], f32, tag="z")
            nc.gpsimd.ap_gather(
                z[:, 0:LPC_PAD], tab[:],
                idx_sb[:, COL_OFFS[h]:COL_OFFS[h] + COLS_HS[h]],
                channels=128, num_elems=2 * NV, d=1, num_idxs=LPC_PAD)

            p01 = work.tile([128, max(CPGS)], f32, tag="p01")
            nc.vector.tensor_tensor(p01[:, 0:CPG], z[:, 0:LPC:3],
                                    z[:, 1:LPC:3], OP.mult)
            r = work.tile([128, max(CPGS)], f32, tag="r")
            # r = ((p01 * -1) * z2) + 1 = 1 - z0 z1 z2
            nc.vector.scalar_tensor_tensor(r[:, 0:CPG], p01[:, 0:CPG],
                                           -1.0, z[:, 2:LPC:3],
                                           OP.mult, OP.mult)
            nc.vector.tensor_scalar_add(r[:, 0:CPG], r[:, 0:CPG], 1.0)

            # PE broadcast: group g's (16-replicated) row -> all 128
            # partitions.  sum over the 16 identical values * 1/16 is
            # bitwise exact.
            P = psum.tile([128, GROUPS, PBLK], f32, tag="P")
            for g in range(GROUPS):
                nc.tensor.matmul(P[:, g, 0:CPG], sel[:, g, :],
                                 r[:, 0:CPG], start=True, stop=True)
            # pack the 8 group blocks contiguously so output descriptors
            # are C_CHUNK*4 bytes
            bcast = work.tile([128, GROUPS * max(CPGS)], f32, tag="bcast")
            bt = bcast[:]
            prow = bt.ap[0][0]
            bview = bass.AP(tensor=bt.tensor, offset=bt.offset,
                            ap=[[prow, 128], [CPG, GROUPS], [1, CPG]])
            nc.scalar.activation(bview, P[:, :, 0:CPG], AF.Copy)

            # 8 row-block output DMAs, 128 rows each, spread across both
            # HWDGE rings
            bap = bass.AP(tensor=bt.tensor, offset=bt.offset,
                          ap=[[prow, 128], [1, C_CHUNK]])
            for blk in range(8):
                dst = bass.AP(tensor=out_d,
                              offset=blk * 128 * C_PAD + C_OFF,
                              ap=[[C_PAD, 128], [1, C_CHUNK]])
                rings[blk % 2].dma_start(out=dst, in_=bap)
    nc.compile()
    return nc


def _prep_indices(clause_idx, clause_sign):
    """Per-core wrapped int16 combined-index arrays [128, IDX_COLS].

    Literal order per group g: chunk-major — for chunk h, group g owns
    core clauses [C_CHUNK*h + CPG*g, C_CHUNK*h + CPG*(g+1)), padded to
    LPC_PAD literals per (group, chunk) block.
    """
    idx2 = clause_idx.astype(np.int32) + NV * (clause_sign <= 0.0)
    idx2 = idx2.astype(np.int16)
    per_core = []
    for c in range(NCORES):
        cl = idx2[c * C_CORE:(c + 1) * C_CORE]            # [5250, 3]
        buf = np.zeros((C_PAD, KLIT), dtype=np.int16)
        buf[:cl.shape[0]] = cl
        # group g's stream = concat over chunks of its padded block
        gs = np.zeros((GROUPS, IDX_COLS * 16), dtype=np.int16)
        for h in range(H):
            blk = buf[C_OFFS[h]:C_OFFS[h] + C_CHUNKS[h]]  # [8*CPG, 3]
            blk = blk.reshape(GROUPS, LPCS[h])
            o = COL_OFFS[h] * 16
            gs[:, o:o + LPCS[h]] = blk
        # wrap: literal j at partition 16g + j%16, col j//16
        w = (gs.reshape(GROUPS, IDX_COLS, 16)
               .transpose(0, 2, 1)
               .reshape(128, IDX_COLS))
        per_core.append(np.ascontiguousarray(w))
    return per_core


def _ensure_ntff_hook():
    """The agent image lacks antenv.axon_hooks; synthesize it so
    run_bass_kernel_spmd(trace=True) can capture NTFF profiles."""
    import sys, types
    try:
        from antenv import axon_hooks  # noqa: F401
        return
    except ImportError:
        pass
    m = types.ModuleType("antenv.axon_hooks")
    _hook = [None]
    m.set_axon_ntff_profile_hook = lambda h: _hook.__setitem__(0, h)
    m.get_axon_ntff_profile_hook = lambda: _hook[0]
    sys.modules["antenv.axon_hooks"] = m
    import antenv
    antenv.axon_hooks = m
    from trn_agent_boot.trn_boot import _ntff_profile_via_ctypes
    m.set_axon_ntff_profile_hook(
        _ntff_profile_via_ctypes("/opt/axon/libaxon_pjrt.so"))


def _run(emb, idx_cores, trace=False):
    from concourse.bass_utils import run_bass_kernel_spmd
    if trace:
        _ensure_ntff_hook()
    if "prog" not in _CACHE:
        _CACHE["prog"] = _build()
    nc = _CACHE["prog"]
    in_maps = [{"emb": emb, "idxw": idx_cores[c]} for c in range(NCORES)]
    return run_bass_kernel_spmd(nc, in_maps, list(range(NCORES)),
                                trace=trace)


def kernel(input_idx=None, emb_weight=None, clause_idx=None,
           clause_sign=None, _trace=False, _want_results=False):
    emb = np.ascontiguousarray(np.asarray(emb_weight, dtype=np.float32))
    cidx = np.asarray(clause_idx, dtype=np.int32)
    csgn = np.asarray(clause_sign, dtype=np.float32)
    idx_cores = _prep_indices(cidx, csgn)
    res = _run(emb, idx_cores, trace=_trace)
    full = np.empty((B, C_TOTAL), dtype=np.float32)
    for c in range(NCORES):
        full[:, c * C_CORE:(c + 1) * C_CORE] = \
            res.results[c]["out"][:, :C_CORE]
    if _want_results:
        return full, res
    return full


# revision 29
# speedup vs baseline: 1.1183x; 1.0695x over previous
"""Trainium2 Bass kernel: batched soft 3-SAT circuit evaluation.

out[b, c] = 1 - prod_k z[c,k],  z = (sign>0 ? 1-x : x)[idx],
x = sigmoid(emb[0]).  Every batch row is identical (input_idx is all
zeros, the embedding has a single row, and jnp.take clamps OOB), so the
device computes each clause result once and broadcast-writes the rows.

Sharding: clauses split across 8 NeuronCores (5250 each, padded 5376).
Host work is index-layout prep only (fold sign into a combined table
index, pad, order literals chunk-major, wrap into the 16-partition
GPSIMD gather layout) plus concatenation of per-core outputs.

Per-core device pipeline (H = 4 column chunks of 1344 cols):
  prologue (4 col-quarters, two HWDGE rings): broadcast-load emb row
    into raw[128, NV]; ACT sigmoid -> x table half; DVE (x*-1)+1 ->
    1-x table half.  Combined table tab[128, 2*NV].
  per chunk h:
    - GPSIMD ap_gather: z[128, 512] literals (8 Q7 groups x 168 clauses)
    - DVE: r = 1 - z0*z1*z2  [128, 168] (replicated within each
      16-partition group)
    - PE: per group g a [K=16]x[M=128]x[N=168] matmul with lhsT=1/16
      broadcasts group g's row into all 128 partitions of PSUM (bitwise
      exact: sum of 16 identical values * 1/16)
    - ACT: copy PSUM -> SBUF bcast tile [128, 8*168]
    - 8 row-block DMAs bcast -> out[128b:128b+128, 1344h:1344h+1344]
      (5.4KB descriptors), alternating the sync/scalar HWDGE rings.
"""

import numpy as np

NV = 10000
C_TOTAL = 42000
KLIT = 3
B = 1024
NCORES = 8
C_CORE = C_TOTAL // NCORES     # 5250
GROUPS = 8                     # Q7 cores / 16-partition groups
C_PAD = 5376                   # padded clauses per core
CPGS = [168, 168, 168, 168]    # clauses per (group, chunk)
H = len(CPGS)
C_CHUNKS = [8 * c for c in CPGS]          # output cols per chunk
C_OFFS = [sum(C_CHUNKS[:h]) for h in range(H)]
LPCS = [c * KLIT for c in CPGS]           # real literals per (g, chunk)
LPC_PADS = [-(-l // 32) * 32 for l in LPCS]   # pad to 32 (2-col align)
COLS_HS = [l // 16 for l in LPC_PADS]     # idx cols per chunk
COL_OFFS = [sum(COLS_HS[:h]) for h in range(H)]
IDX_COLS = sum(COLS_HS)
PBLK = 256                     # PSUM cols reserved per group block

_CACHE = {}


def _build():
    import concourse.bass as bass
    import concourse.tile as tile
    from concourse import bacc, mybir
    from contextlib import ExitStack

    f32 = mybir.dt.float32
    AF = mybir.ActivationFunctionType
    OP = mybir.AluOpType

    nc = bacc.Bacc("TRN2", target_bir_lowering=False, debug=False,
                   num_devices=NCORES)
    emb_d = nc.dram_tensor("emb", [1, NV], f32, kind="ExternalInput")
    idx_d = nc.dram_tensor("idxw", [128, IDX_COLS], mybir.dt.int16,
                           kind="ExternalInput")
    out_d = nc.dram_tensor("out", [B, C_PAD], f32, kind="ExternalOutput")

    with tile.TileContext(nc) as tc, ExitStack() as ctx:
        const = ctx.enter_context(tc.tile_pool(name="const", bufs=1))
        work = ctx.enter_context(tc.tile_pool(name="work", bufs=2))
        psum = ctx.enter_context(
            tc.tile_pool(name="psum", bufs=2, space="PSUM"))

        idx_sb = const.tile([128, IDX_COLS], mybir.dt.int16)

        # selector E[:, g, :]: E[k, g, m] = 1/16 iff k//16 == g; matmul
        # with it averages each group's 16 identical partition rows into
        # all 128 output partitions (bitwise exact).
        sel = const.tile([128, GROUPS, 128], f32)
        nc.vector.memset(sel[:], 1.0 / 16.0)
        # keep 1/16 only where 0 <= p - 16g <= 15, i.e. g == p//16
        nc.gpsimd.affine_select(sel[:, :, :], sel[:, :, :],
                                pattern=[[-16, GROUPS], [0, 128]],
                                compare_op=OP.is_ge, fill=0.0,
                                base=0, channel_multiplier=1)
        nc.gpsimd.affine_select(sel[:, :, :], sel[:, :, :],
                                pattern=[[16, GROUPS], [0, 128]],
                                compare_op=OP.is_ge, fill=0.0,
                                base=15, channel_multiplier=-1)

        raw = const.tile([128, NV], f32)
        tab = const.tile([128, 2 * NV], f32)
        rings = [nc.sync, nc.scalar]
        NQ = 8
        q = NV // NQ
        # broadcast-load eighths alternate sync HWDGE / gpsimd SWDGE:
        # two queues give aggregate HBM-read rate, and the scalar ring
        # stays clear so ACT's sequencer isn't delayed by DMA dispatch
        for c in range(NQ):
            eng = nc.sync if c % 2 == 0 else nc.gpsimd
            eng.dma_start(
                out=raw[:, c * q:(c + 1) * q],
                in_=bass.AP(tensor=emb_d, offset=c * q,
                            ap=[[0, 128], [1, q]]))
        nc.gpsimd.dma_start(out=idx_sb[:], in_=idx_d[:, :])
        for c in range(NQ):
            sl = slice(c * q, (c + 1) * q)
            xs = slice(NV + c * q, NV + (c + 1) * q)
            nc.scalar.activation(tab[:, xs], raw[:, sl], AF.Sigmoid)
            # 1 - x on DVE, overlaps ACT of the next eighth
            nc.vector.tensor_scalar(tab[:, sl], tab[:, xs], -1.0, 1.0,
                                    OP.mult, OP.add)

        for h in range(H):
            CPG, LPC, LPC_PAD = CPGS[h], LPCS[h], LPC_PADS[h]
            C_CHUNK, C_OFF = C_CHUNKS[h], C_OFFS[h]
            z = work.tile([128, max(LPC_PADS)], f32, tag="z")
            nc.gpsimd.ap_gather(
                z[:, 0:LPC_PAD], tab[:],
                idx_sb[:, COL_OFFS[h]:COL_OFFS[h] + COLS_HS[h]],
                channels=128, num_elems=2 * NV, d=1, num_idxs=LPC_PAD)

            p01 = work.tile([128, max(CPGS)], f32, tag="p01")
            nc.vector.tensor_tensor(p01[:, 0:CPG], z[:, 0:LPC:3],
                                    z[:, 1:LPC:3], OP.mult)
            r = work.tile([128, max(CPGS)], f32, tag="r")
            # r = z0 z1 z2 (the 1 - . fold happens in the ACT copy)
            nc.vector.scalar_tensor_tensor(r[:, 0:CPG], p01[:, 0:CPG],
                                           1.0, z[:, 2:LPC:3],
                                           OP.mult, OP.mult)

            # PE broadcast: group g's (16-replicated) row -> all 128
            # partitions.  sum over the 16 identical values * 1/16 is
            # bitwise exact.
            P = psum.tile([128, GROUPS, PBLK], f32, tag="P")
            for g in range(GROUPS):
                nc.tensor.matmul(P[:, g, 0:CPG], sel[:, g, :],
                                 r[:, 0:CPG], start=True, stop=True)
            # pack the 8 group blocks contiguously so output descriptors
            # are C_CHUNK*4 bytes
            bcast = work.tile([128, GROUPS * max(CPGS)], f32, tag="bcast")
            bt = bcast[:]
            prow = bt.ap[0][0]
            bview = bass.AP(tensor=bt.tensor, offset=bt.offset,
                            ap=[[prow, 128], [CPG, GROUPS], [1, CPG]])
            # bcast = Copy(-P + 1) = 1 - z0 z1 z2
            nc.scalar.activation(bview, P[:, :, 0:CPG], AF.Copy,
                                 scale=-1.0, bias=1.0)

            # 8 row-block output DMAs, 128 rows each, spread across both
            # HWDGE rings
            bap = bass.AP(tensor=bt.tensor, offset=bt.offset,
                          ap=[[prow, 128], [1, C_CHUNK]])
            for blk in range(8):
                dst = bass.AP(tensor=out_d,
                              offset=blk * 128 * C_PAD + C_OFF,
                              ap=[[C_PAD, 128], [1, C_CHUNK]])
                rings[blk % 2].dma_start(out=dst, in_=bap)
    nc.compile()
    return nc


def _prep_indices(clause_idx, clause_sign):
    """Per-core wrapped int16 combined-index arrays [128, IDX_COLS].

    Literal order per group g: chunk-major — for chunk h, group g owns
    core clauses [C_CHUNK*h + CPG*g, C_CHUNK*h + CPG*(g+1)), padded to
    LPC_PAD literals per (group, chunk) block.
    """
    idx2 = clause_idx.astype(np.int32) + NV * (clause_sign <= 0.0)
    idx2 = idx2.astype(np.int16)
    per_core = []
    for c in range(NCORES):
        cl = idx2[c * C_CORE:(c + 1) * C_CORE]            # [5250, 3]
        buf = np.zeros((C_PAD, KLIT), dtype=np.int16)
        buf[:cl.shape[0]] = cl
        # group g's stream = concat over chunks of its padded block
        gs = np.zeros((GROUPS, IDX_COLS * 16), dtype=np.int16)
        for h in range(H):
            blk = buf[C_OFFS[h]:C_OFFS[h] + C_CHUNKS[h]]  # [8*CPG, 3]
            blk = blk.reshape(GROUPS, LPCS[h])
            o = COL_OFFS[h] * 16
            gs[:, o:o + LPCS[h]] = blk
        # wrap: literal j at partition 16g + j%16, col j//16
        w = (gs.reshape(GROUPS, IDX_COLS, 16)
               .transpose(0, 2, 1)
               .reshape(128, IDX_COLS))
        per_core.append(np.ascontiguousarray(w))
    return per_core


def _ensure_ntff_hook():
    """The agent image lacks antenv.axon_hooks; synthesize it so
    run_bass_kernel_spmd(trace=True) can capture NTFF profiles."""
    import sys, types
    try:
        from antenv import axon_hooks  # noqa: F401
        return
    except ImportError:
        pass
    m = types.ModuleType("antenv.axon_hooks")
    _hook = [None]
    m.set_axon_ntff_profile_hook = lambda h: _hook.__setitem__(0, h)
    m.get_axon_ntff_profile_hook = lambda: _hook[0]
    sys.modules["antenv.axon_hooks"] = m
    import antenv
    antenv.axon_hooks = m
    from trn_agent_boot.trn_boot import _ntff_profile_via_ctypes
    m.set_axon_ntff_profile_hook(
        _ntff_profile_via_ctypes("/opt/axon/libaxon_pjrt.so"))


def _run(emb, idx_cores, trace=False):
    from concourse.bass_utils import run_bass_kernel_spmd
    if trace:
        _ensure_ntff_hook()
    if "prog" not in _CACHE:
        _CACHE["prog"] = _build()
    nc = _CACHE["prog"]
    in_maps = [{"emb": emb, "idxw": idx_cores[c]} for c in range(NCORES)]
    return run_bass_kernel_spmd(nc, in_maps, list(range(NCORES)),
                                trace=trace)


def kernel(input_idx=None, emb_weight=None, clause_idx=None,
           clause_sign=None, _trace=False, _want_results=False):
    emb = np.ascontiguousarray(np.asarray(emb_weight, dtype=np.float32))
    cidx = np.asarray(clause_idx, dtype=np.int32)
    csgn = np.asarray(clause_sign, dtype=np.float32)
    idx_cores = _prep_indices(cidx, csgn)
    res = _run(emb, idx_cores, trace=_trace)
    full = np.empty((B, C_TOTAL), dtype=np.float32)
    for c in range(NCORES):
        full[:, c * C_CORE:(c + 1) * C_CORE] = \
            res.results[c]["out"][:, :C_CORE]
    if _want_results:
        return full, res
    return full
